# revision 11
# baseline (speedup 1.0000x reference)
"""Multi-head causal attention (B=8,S=1024,D=768,H=12,Dh=64) on 8 TRN2 NeuronCores.

Data-parallel over batch: each core handles one batch element end-to-end.
No collectives.

Precision (rel-err budget 2e-2; this config measures ~9.8e-3 on CPU):
  - Q/K projections run fp8 e4m3 with MatmulPerfMode.DoubleRow: two 128-deep
    contraction tiles per instruction -> 2x MAC rate vs bf16 (measured on HW:
    DoubleRow streams ~1 cycle/output-col like bf16, with doubled contraction).
  - Scores / V / z / l / output projection stay bf16 (fp8 there fails the
    error budget; zero-padded DoubleRow scores gain nothing on real HW).
  - x is shipped as fp8 + fp8 residual and reconstructed to bf16 on-chip
    (DVE add) for the V path - fewer HBM bytes, more accurate than bf16 x.

Schedule: ACT (exp, ~65us) is the steady-state bottleneck. Scores for the
next burst are emitted ahead of each z/l so ACT never starves; V is
interleaved block-wise with lookahead scores early on; qh=1 (the 8-block
half) is processed before qh=0 per pair so the tail ends on the small qh=0
unit; output projection i4-7 rides the final ACT bursts, i0-3 is the tail.
PE warmup matmuls ramp the tensor-engine p-state during the DMA window.
DMAs are consolidated (partition-major host packing) and ordered by need.

Per-core layout:
  x8/r8    [128(d), 3(dtp), 2(sub), 1024(s)] fp8  x^T fp8 + residual
  xTb[dt]  [128(d), 1024(s)] bf16                 x^T = x8 + r8 (DVE)
  qT/kT[p] [128(he-pair), 1024(s)] bf16           head-pair packed
  v[kt]    [128(s), 768(h e)] bf16
  zT[p]    [128(he-pair), 1024(s)] bf16
  Scores transposed (S^T[k,q]); softmax sum over k via ones-matmul on PE
  (paired with z in disjoint PE column groups); causal diag mask via
  gpsimd affine_select on the exp'd pattern.
"""
import sys

sys.path.insert(0, "/opt/trn_rl_repo")

import numpy as np

import concourse.bacc as bacc
import concourse.mybir as mybir
from concourse import tile
from concourse import bass_utils
from concourse.bass_interp import get_hw_module

F32 = mybir.dt.float32
BF16 = mybir.dt.bfloat16
FP8 = mybir.dt.float8e4
EXP = mybir.ActivationFunctionType.Exp
DR = mybir.MatmulPerfMode.DoubleRow

B, S, D, H, Dh = 8, 1024, 768, 12, 64
NP = 128
DT = D // NP      # 6 d-tiles
DTP = DT // 2     # 3 d-tile pairs (DoubleRow)
ST = S // NP
KT = S // NP
NPAIR = H // 2
SCALE = 1.0 / 8.0
N_WARM = 14


def _build(debug=False):
    nc = bacc.Bacc(
        "TRN2",
        target_bir_lowering=False,
        debug=False,
        enable_asserts=False,
        num_devices=8,
    )
    x8_d = nc.dram_tensor("x8", (NP, DTP, 2, S), FP8, kind="ExternalInput")
    r8_d = nc.dram_tensor("r8", (NP, DTP, 2, S), FP8, kind="ExternalInput")
    wq_d = nc.dram_tensor("wq8", (NP, NPAIR, DTP, 2, NP), FP8, kind="ExternalInput")
    wk_d = nc.dram_tensor("wk8", (NP, NPAIR, DTP, 2, NP), FP8, kind="ExternalInput")
    wv_d = nc.dram_tensor("wv", (NP, DT, H * Dh), BF16, kind="ExternalInput")
    wo_d = nc.dram_tensor("wo", (NP, NPAIR, D), BF16, kind="ExternalInput")
    bq_d = nc.dram_tensor("bq", (H, Dh), F32, kind="ExternalInput")
    bk_d = nc.dram_tensor("bk", (H, Dh), F32, kind="ExternalInput")
    bv_d = nc.dram_tensor("bv", (H, Dh), F32, kind="ExternalInput")
    bo_d = nc.dram_tensor("bo", (D,), F32, kind="ExternalInput")
    out_d = nc.dram_tensor("out", (S, D), BF16, kind="ExternalOutput")
    dbg = {}
    if debug:
        dbg["qT"] = nc.dram_tensor("dbg_qT", (NPAIR, NP, S), BF16, kind="ExternalOutput")
        dbg["kT"] = nc.dram_tensor("dbg_kT", (NPAIR, NP, S), BF16, kind="ExternalOutput")
        dbg["v"] = nc.dram_tensor("dbg_v", (KT, NP, H * Dh), BF16, kind="ExternalOutput")
        dbg["zT"] = nc.dram_tensor("dbg_zT", (NPAIR, NP, S), BF16, kind="ExternalOutput")
        dbg["xTb"] = nc.dram_tensor("dbg_xTb", (DT, NP, S), BF16, kind="ExternalOutput")

    with tile.TileContext(nc) as tc:
        _body(tc, x8_d, r8_d, wq_d, wk_d, wv_d, wo_d, bq_d, bk_d, bv_d, bo_d,
              out_d, dbg)

    nc.compile()
    return nc


def _body(tc, x8_d, r8_d, wq_d, wk_d, wv_d, wo_d, bq_d, bk_d, bv_d, bo_d,
          out_d, dbg):
    nc = tc.nc

    with (
        tc.tile_pool(name="const", bufs=1) as const_pool,
        tc.tile_pool(name="qkT", bufs=1) as qkT_pool,
        tc.tile_pool(name="vsb", bufs=1) as v_pool,
        tc.tile_pool(name="zT", bufs=1) as zT_pool,
        tc.tile_pool(name="xT", bufs=1) as xT_pool,
        tc.tile_pool(name="w", bufs=1) as w_pool,
        tc.tile_pool(name="pt", bufs=30) as pt_pool,
        tc.tile_pool(name="rcp", bufs=2) as r_pool,
        tc.tile_pool(name="osb", bufs=3) as o_pool,
    ):
        # ---- constants (no DMA deps) ----
        warm = const_pool.tile([NP, NP], BF16, tag="warm")
        nc.gpsimd.memset(warm[:], 1.0)
        warm2 = const_pool.tile([NP, 256], BF16, tag="warm2")
        nc.gpsimd.memset(warm2[:], 1.0)
        ones64 = const_pool.tile([NP, 64], BF16, tag="ones64")
        nc.gpsimd.memset(ones64[:], 1.0)
        bq_sb = const_pool.tile([NP, NPAIR], F32, tag="bq")
        bk_sb = const_pool.tile([NP, NPAIR], F32, tag="bk")
        bv_rep = const_pool.tile([NP, H * Dh], F32, tag="bvrep")
        bo_rep = const_pool.tile([NP, D], F32, tag="borep")
        fill0 = nc.gpsimd.to_reg(0.0)

        # ---- persistent tiles ----
        qT = [qkT_pool.tile([NP, S], BF16, tag=f"qT{p}", name=f"qT{p}")
              for p in range(NPAIR)]
        kT = [qkT_pool.tile([NP, S], BF16, tag=f"kT{p}", name=f"kT{p}")
              for p in range(NPAIR)]
        v_sb = [v_pool.tile([NP, H * Dh], BF16, tag=f"v{k}", name=f"v{k}")
                for k in range(KT)]
        zT = [zT_pool.tile([NP, S], BF16, tag=f"zT{p}", name=f"zT{p}")
              for p in range(NPAIR)]
        x8_sb = xT_pool.tile([NP, DTP, 2, S], FP8, tag="x8", name="x8_sb")
        r8_sb = xT_pool.tile([NP, DTP, 2, S], FP8, tag="r8", name="r8_sb")
        xTb = [xT_pool.tile([NP, S], BF16, tag=f"xb{t}", name=f"xb{t}")
               for t in range(DT)]
        wk_sb = w_pool.tile([NP, NPAIR, DTP, 2, NP], FP8, tag="wk", name="wk_sb")
        wq_sb = w_pool.tile([NP, NPAIR, DTP, 2, NP], FP8, tag="wq", name="wq_sb")
        wv_sb = w_pool.tile([NP, DT, H * Dh], BF16, tag="wv", name="wv_sb")
        wo_sb = w_pool.tile([NP, NPAIR, D], BF16, tag="wo", name="wo_sb")

        # ---- DMA issues (per queue = priority order; all partition-major) ----
        # x8 gates everything: split it 3 ways so QK can start ~9us
        nc.sync.dma_start(x8_sb[:, 0, 0], x8_d.ap()[:, 0, 0])
        nc.scalar.dma_start(x8_sb[:, 0, 1], x8_d.ap()[:, 0, 1])
        nc.gpsimd.dma_start(x8_sb[:, 1, 0], x8_d.ap()[:, 1, 0])
        nc.sync.dma_start(x8_sb[:, 1, 1], x8_d.ap()[:, 1, 1])
        nc.scalar.dma_start(x8_sb[:, 2, 0], x8_d.ap()[:, 2, 0])
        nc.gpsimd.dma_start(x8_sb[:, 2, 1], x8_d.ap()[:, 2, 1])
        # sync: wk p0 (QK0), then wv halves
        nc.sync.dma_start(wk_sb[:, 0:1], wk_d.ap()[:, 0:1])
        nc.sync.dma_start(wv_sb[:, 0:3, :], wv_d.ap()[:, 0:3, :])
        nc.sync.dma_start(wv_sb[:, 3:6, :], wv_d.ap()[:, 3:6, :])
        # scalar: wk rest (QK1+), r8 (xTb/V)
        nc.scalar.dma_start(wk_sb[:, 1:NPAIR], wk_d.ap()[:, 1:NPAIR])
        nc.scalar.dma_start(r8_sb[:], r8_d.ap())
        # gpsimd: small biases + wq p0, bv broadcast, wq rest, wo, bo
        nc.gpsimd.dma_start(
            bq_sb[:], bq_d.ap().rearrange("h e -> (h e)").rearrange("(j p) -> p j", p=NP)
        )
        nc.gpsimd.dma_start(
            bk_sb[:], bk_d.ap().rearrange("h e -> (h e)").rearrange("(j p) -> p j", p=NP)
        )
        nc.gpsimd.dma_start(wq_sb[:, 0:1], wq_d.ap()[:, 0:1])
        nc.gpsimd.dma_start(
            bv_rep[:],
            bv_d.ap().rearrange("h e -> (h e)").unsqueeze(0).broadcast_to((NP, H * Dh)),
        )
        nc.gpsimd.dma_start(wq_sb[:, 1:NPAIR], wq_d.ap()[:, 1:NPAIR])
        nc.gpsimd.dma_start(wo_sb[:], wo_d.ap())
        nc.gpsimd.dma_start(bo_rep[:], bo_d.ap().unsqueeze(0).broadcast_to((NP, D)))

        with (
            tc.tile_pool(name="psQK", bufs=2, space="PSUM") as psQK,
            tc.tile_pool(name="psS", bufs=2, space="PSUM") as psS,
            tc.tile_pool(name="psZ", bufs=1, space="PSUM") as psZ,
            tc.tile_pool(name="psL", bufs=1, space="PSUM") as psL,
        ):
            # ---- PE warmup: ramp p-state while DMAs land ----
            wps = psQK.tile([NP, 512], F32, tag="qk", name="warmps")
            for i in range(N_WARM):
                nc.tensor.matmul(wps[:, 0:256], warm[:], warm2[:],
                                 start=True, stop=True, skip_group_check=True)

            def emit_qk_group(p, which):
                """Q or K projection for pair p: fp8 DoubleRow, 3 dtp x 2 sc,
                bf16 output with bias."""
                w_sb, b_sb, dstT = (
                    (wk_sb, bk_sb, kT) if which == 0 else (wq_sb, bq_sb, qT)
                )
                pss = [psQK.tile([NP, 512], F32, tag="qk",
                                 name=f"qk{p}_{which}_{sc}") for sc in range(2)]
                for tp in range(DTP):
                    lhs = w_sb[:, p, tp, :, :]
                    for sc in range(2):
                        nc.tensor.matmul(
                            pss[sc][:], lhs, x8_sb[:, tp, :, sc * 512:(sc + 1) * 512],
                            start=(tp == 0), stop=(tp == DTP - 1), perf_mode=DR,
                        )
                for sc in range(2):
                    nc.vector.tensor_scalar_add(
                        dstT[p][:, sc * 512:(sc + 1) * 512], pss[sc][:],
                        b_sb[:, p:p + 1]
                    )

            def block_geom(qh, kt):
                qlo = qh * 512
                q0 = kt * NP
                c0 = max(q0, qlo)
                w = qlo + 512 - c0
                return qlo, q0, c0, w

            def emit_score_block(p, qh, kt, pts):
                """S^T (bf16) + exp (+ causal diag mask) for one block."""
                qlo, q0, c0, w = block_geom(qh, kt)
                st = psS.tile([NP, 2, 512], F32, tag="st", name="st")
                for h in range(2):
                    nc.tensor.matmul(
                        st[:, h, 0:w],
                        kT[p][h * 64:(h + 1) * 64, q0:q0 + NP],
                        qT[p][h * 64:(h + 1) * 64, c0:c0 + w],
                        start=True, stop=True,
                    )
                pt = pt_pool.tile([NP, 2, 512], BF16, tag="pt", name="pt")
                nc.scalar.activation(pt[:, :, 0:w], st[:, :, 0:w], EXP, scale=SCALE)
                if c0 == q0:  # diagonal block: zero out k > q
                    nc.gpsimd.affine_select(
                        out=pt[:, 0:2, 0:NP], in_=pt[:, 0:2, 0:NP],
                        pattern=[[0, 2], [1, NP]],
                        compare_op=mybir.AluOpType.is_ge,
                        fill=fill0, base=0, channel_multiplier=-1,
                    )
                pts[kt] = (pt, c0, w)

            def block_list(qh):
                return list(range(4)) if qh == 0 else list(range(KT))

            def emit_scores(p, qh, pts):
                for kt in block_list(qh):
                    emit_score_block(p, qh, kt, pts)

            def emit_zl(p, qh, pts, filler=None):
                qlo = qh * 512
                z_ps = psZ.tile([NP, 512], F32, tag="z", name="z")
                l_ps = psL.tile([NP, 512], F32, tag="l", name="l")
                kts = block_list(qh)
                for kt in kts:
                    if filler is not None:
                        filler()
                    pt, c0, w = pts.pop(kt)
                    first = kt == kts[0]
                    last = kt == kts[-1]

                    def mm_l(h):
                        nc.tensor.matmul(
                            l_ps[h * 64:(h + 1) * 64, c0 - qlo:c0 - qlo + w],
                            ones64[:, 0:64], pt[:, h, 0:w],
                            start=first, stop=last, skip_group_check=True,
                        )

                    def mm_z(h):
                        nc.tensor.matmul(
                            z_ps[h * 64:(h + 1) * 64, c0 - qlo:c0 - qlo + w],
                            v_sb[kt][:, (2 * p + h) * 64:(2 * p + h + 1) * 64],
                            pt[:, h, 0:w],
                            start=first, stop=last, skip_group_check=True,
                        )
                    mm_l(0); mm_z(1); mm_l(1); mm_z(0)
                recip = r_pool.tile([NP, 512], F32, tag="rcp", name="recip")
                nc.vector.reciprocal_approx_fast(out=recip[:], in_=l_ps[:])
                nc.vector.tensor_mul(zT[p][:, qlo:qlo + 512], z_ps[:], recip[:])

            def emit_v(kt):
                ps1 = psQK.tile([NP, 512], F32, tag="qk", name=f"v{kt}a")
                ps2 = psQK.tile([NP, 512], F32, tag="qk", name=f"v{kt}b")
                for t in range(DT):
                    lhs = xTb[t][:, kt * NP:(kt + 1) * NP]
                    nc.tensor.matmul(ps1[:], lhs, wv_sb[:, t, 0:512],
                                     start=(t == 0), stop=(t == DT - 1))
                    nc.tensor.matmul(ps2[:, 0:256], lhs, wv_sb[:, t, 512:768],
                                     start=(t == 0), stop=(t == DT - 1))
                nc.vector.tensor_add(v_sb[kt][:, 0:512], ps1[:], bv_rep[:, 0:512])
                nc.vector.tensor_add(v_sb[kt][:, 512:768], ps2[:, 0:256],
                                     bv_rep[:, 512:768])

            def emit_outproj(i):
                ps1 = psQK.tile([NP, 512], F32, tag="qk", name=f"op{i}a")
                ps2 = psQK.tile([NP, 512], F32, tag="qk", name=f"op{i}b")
                for p2 in range(NPAIR):
                    lhs = zT[p2][:, i * NP:(i + 1) * NP]
                    nc.tensor.matmul(ps1[:], lhs, wo_sb[:, p2, 0:512],
                                     start=(p2 == 0), stop=(p2 == NPAIR - 1))
                    nc.tensor.matmul(ps2[:, 0:256], lhs, wo_sb[:, p2, 512:768],
                                     start=(p2 == 0), stop=(p2 == NPAIR - 1))
                o_t = o_pool.tile([NP, D], BF16, tag="o", name=f"ot{i}")
                nc.vector.tensor_add(o_t[:, 0:512], ps1[:], bo_rep[:, 0:512])
                nc.vector.tensor_add(o_t[:, 512:768], ps2[:, 0:256], bo_rep[:, 512:768])
                nc.sync.dma_start(out_d.ap()[i * NP:(i + 1) * NP, :], o_t[:])

            # ================= schedule =================
            # Block-granular software pipeline: a filler queue of future
            # score blocks / QK groups is woven one item per z/l step so the
            # ACT engine always has exp work queued.
            pts = {(p, qh): {} for p in range(NPAIR) for qh in range(2)}

            fill_q = []
            for p in range(1, NPAIR):
                if p >= 2:
                    fill_q.append(('qk', p, 0))
                    fill_q.append(('qk', p, 1))
                for qh in range(2):
                    for kt in block_list(qh):
                        fill_q.append(('s', p, qh, kt))

            def filler():
                if not fill_q:
                    return
                item = fill_q.pop(0)
                if item[0] == 'qk':
                    emit_qk_group(item[1], item[2])
                else:
                    _, p_, qh_, kt_ = item
                    emit_score_block(p_, qh_, kt_, pts[(p_, qh_)])

            emit_qk_group(0, 0)
            emit_qk_group(0, 1)
            emit_scores(0, 0, pts[(0, 0)])
            emit_scores(0, 1, pts[(0, 1)])
            emit_qk_group(1, 0)
            emit_qk_group(1, 1)
            # reconstruct bf16 x for the V path; split gpsimd/DVE queues
            for t in range(DT):
                tp, sub = divmod(t, 2)
                eng = nc.gpsimd if t % 2 == 0 else nc.vector
                eng.tensor_add(xTb[t][:], x8_sb[:, tp, sub, :],
                               r8_sb[:, tp, sub, :])

            for kt in range(KT):
                emit_v(kt)
                filler()

            for p in range(NPAIR):
                for qh in range(2):
                    if (p, qh) == (NPAIR - 1, 1):
                        # drain any remaining scores first, then let the
                        # first outproj half ride the final ACT bursts
                        while fill_q:
                            filler()
                        for i in range(4):
                            emit_outproj(i)
                    emit_zl(p, qh, pts[(p, qh)], filler)

        # ---- output projection second half (dedicated PSUM, pipelined) ----
        with tc.tile_pool(name="psO", bufs=3, space="PSUM") as psO:
            for i in range(4, ST):
                ps = psO.tile([NP, 1024], F32, tag="o", name="psO")
                for p2 in range(NPAIR):
                    lhs = zT[p2][:, i * NP:(i + 1) * NP]
                    nc.tensor.matmul(ps[:, 0:512], lhs, wo_sb[:, p2, 0:512],
                                     start=(p2 == 0), stop=(p2 == NPAIR - 1))
                    nc.tensor.matmul(ps[:, 512:768], lhs, wo_sb[:, p2, 512:768],
                                     start=(p2 == 0), stop=(p2 == NPAIR - 1))
                o_t = o_pool.tile([NP, D], BF16, tag="o", name=f"ot{i}")
                # split bias/DMA so the store of the first half overlaps the
                # bias-add of the second on the final tiles
                nc.vector.tensor_add(o_t[:, 0:384], ps[:, 0:384], bo_rep[:, 0:384])
                nc.sync.dma_start(out_d.ap()[i * NP:(i + 1) * NP, 0:384],
                                  o_t[:, 0:384])
                nc.vector.tensor_add(o_t[:, 384:768], ps[:, 384:768],
                                     bo_rep[:, 384:768])
                nc.sync.dma_start(out_d.ap()[i * NP:(i + 1) * NP, 384:768],
                                  o_t[:, 384:768])

        if dbg:
            for p in range(NPAIR):
                nc.sync.dma_start(dbg["qT"].ap()[p], qT[p][:])
                nc.sync.dma_start(dbg["kT"].ap()[p], kT[p][:])
                nc.sync.dma_start(dbg["zT"].ap()[p], zT[p][:])
            for kt in range(KT):
                nc.sync.dma_start(dbg["v"].ap()[kt], v_sb[kt][:])
            for t in range(DT):
                nc.sync.dma_start(dbg["xTb"].ap()[t], xTb[t][:])


_NC = None


def _get_nc():
    global _NC
    if _NC is None:
        nc = _build(debug=False)
        nc.m = get_hw_module(nc.m)
        _NC = nc
    return _NC


def _host_pack(inputs):
    import ml_dtypes
    E4 = ml_dtypes.float8_e4m3
    BF = ml_dtypes.bfloat16

    def pack_x(xb):
        # x [S, D] f32 -> xT [D, S] -> fp8 + fp8 residual, [NP, DTP, 2, S]
        xt = np.ascontiguousarray(xb.T).astype(np.float32)
        x8 = xt.astype(E4)
        r = xt - x8.astype(np.float32)
        r8 = r.astype(E4)

        def fold(a):
            return np.ascontiguousarray(
                a.reshape(DTP, 2, NP, S).transpose(2, 0, 1, 3))
        return fold(x8), fold(r8)

    def pack_wqk(w):
        # [H, D, Dh] -> [D, H*Dh] fp8 -> [NP, NPAIR, DTP, 2, NP]
        wt = np.asarray(w, np.float32).transpose(1, 0, 2).reshape(D, H * Dh)
        w8 = wt.astype(E4)
        a = w8.reshape(DTP, 2, NP, H * Dh).transpose(2, 0, 1, 3)  # [dp, tp, sub, he]
        a = a.reshape(NP, DTP, 2, NPAIR, NP).transpose(0, 3, 1, 2, 4)
        return np.ascontiguousarray(a)

    def pack_wv(w):
        # [H, D, Dh] -> [D, H*Dh] -> [NP(dp), DT, H*Dh] bf16
        wt = np.asarray(w, np.float32).transpose(1, 0, 2).reshape(DT, NP, H * Dh)
        return np.ascontiguousarray(wt.transpose(1, 0, 2)).astype(BF)

    wo = np.asarray(inputs["W_O"], np.float32)
    # [H, Dh, D] -> [NPAIR, NP(he), D] -> [NP(he), NPAIR, D]
    wo_p = np.ascontiguousarray(wo.reshape(NPAIR, NP, D).transpose(1, 0, 2))
    shared = {
        "wq8": pack_wqk(inputs["W_Q"]),
        "wk8": pack_wqk(inputs["W_K"]),
        "wv": pack_wv(inputs["W_V"]),
        "wo": wo_p.astype(BF),
        "bq": np.ascontiguousarray(np.asarray(inputs["b_Q"], np.float32)),
        "bk": np.ascontiguousarray(np.asarray(inputs["b_K"], np.float32)),
        "bv": np.ascontiguousarray(np.asarray(inputs["b_V"], np.float32)),
        "bo": np.ascontiguousarray(np.asarray(inputs["b_O"], np.float32)),
    }
    x = np.asarray(inputs["normalized_resid_pre"], np.float32)
    maps = []
    for b in range(B):
        x8, r8 = pack_x(x[b])
        maps.append(dict(shared, x8=x8, r8=r8))
    return maps


def kernel(**inputs):
    nc = _get_nc()
    res = bass_utils.run_bass_kernel_spmd(nc, _host_pack(inputs),
                                          core_ids=list(range(B)))
    return np.stack([res.results[b]["out"] for b in range(B)],
                    axis=0).astype(np.float32)


def kernel_traced(**inputs):
    """Like kernel() but captures an NTFF profile (ntff shim must be
    installed by the caller). Returns (out, BassKernelResults)."""
    nc = _get_nc()
    res = bass_utils.run_bass_kernel_spmd(
        nc, _host_pack(inputs), core_ids=list(range(B)), trace=True
    )
    out = np.stack([res.results[b]["out"] for b in range(B)],
                   axis=0).astype(np.float32)
    return out, res


# revision 12
# speedup vs baseline: 1.0100x; 1.0100x over previous
"""Multi-head causal attention (B=8,S=1024,D=768,H=12,Dh=64) on 8 TRN2 NeuronCores.

Data-parallel over batch: each core handles one batch element end-to-end.
No collectives.

Precision (rel-err budget 2e-2; this config measures ~9.8e-3 on CPU):
  - Q/K projections run fp8 e4m3 with MatmulPerfMode.DoubleRow: two 128-deep
    contraction tiles per instruction -> 2x MAC rate vs bf16 (measured on HW:
    DoubleRow streams ~1 cycle/output-col like bf16, with doubled contraction).
  - Scores / V / z / l / output projection stay bf16 (fp8 there fails the
    error budget; zero-padded DoubleRow scores gain nothing on real HW).
  - x is shipped as fp8 + fp8 residual and reconstructed to bf16 on-chip
    (DVE add) for the V path - fewer HBM bytes, more accurate than bf16 x.

Schedule: ACT (exp, ~65us) is the steady-state bottleneck. Scores for the
next burst are emitted ahead of each z/l so ACT never starves; V is
interleaved block-wise with lookahead scores early on; qh=1 (the 8-block
half) is processed before qh=0 per pair so the tail ends on the small qh=0
unit; output projection i4-7 rides the final ACT bursts, i0-3 is the tail.
PE warmup matmuls ramp the tensor-engine p-state during the DMA window.
DMAs are consolidated (partition-major host packing) and ordered by need.

Per-core layout:
  x8/r8    [128(d), 3(dtp), 2(sub), 1024(s)] fp8  x^T fp8 + residual
  xTb[dt]  [128(d), 1024(s)] bf16                 x^T = x8 + r8 (DVE)
  qT/kT[p] [128(he-pair), 1024(s)] bf16           head-pair packed
  v[kt]    [128(s), 768(h e)] bf16
  zT[p]    [128(he-pair), 1024(s)] bf16
  Scores transposed (S^T[k,q]); softmax sum over k via ones-matmul on PE
  (paired with z in disjoint PE column groups); causal diag mask via
  gpsimd affine_select on the exp'd pattern.
"""
import sys

sys.path.insert(0, "/opt/trn_rl_repo")

import numpy as np

import concourse.bacc as bacc
import concourse.mybir as mybir
from concourse import tile
from concourse import bass_utils
from concourse.bass_interp import get_hw_module

F32 = mybir.dt.float32
BF16 = mybir.dt.bfloat16
FP8 = mybir.dt.float8e4
EXP = mybir.ActivationFunctionType.Exp
DR = mybir.MatmulPerfMode.DoubleRow
DRSI = mybir.MatmulPerfMode.DoubleRowSwInterleave

B, S, D, H, Dh = 8, 1024, 768, 12, 64
NP = 128
DT = D // NP      # 6 d-tiles
DTP = DT // 2     # 3 d-tile pairs (DoubleRow)
ST = S // NP
KT = S // NP
NPAIR = H // 2
SCALE = 1.0 / 8.0
N_WARM = 14


def _build(debug=False):
    nc = bacc.Bacc(
        "TRN2",
        target_bir_lowering=False,
        debug=False,
        enable_asserts=False,
        num_devices=8,
    )
    x8_d = nc.dram_tensor("x8", (NP, DTP, 2, S), FP8, kind="ExternalInput")
    r8_d = nc.dram_tensor("r8", (NP, DTP, 2, S), FP8, kind="ExternalInput")
    wq_d = nc.dram_tensor("wq8", (NP, NPAIR, DTP, 2, NP), FP8, kind="ExternalInput")
    wk_d = nc.dram_tensor("wk8", (NP, NPAIR, DTP, 2, NP), FP8, kind="ExternalInput")
    wv_d = nc.dram_tensor("wv", (NP, DT, H * Dh), BF16, kind="ExternalInput")
    wo_d = nc.dram_tensor("wo", (NP, NPAIR, D), BF16, kind="ExternalInput")
    bq_d = nc.dram_tensor("bq", (H, Dh), F32, kind="ExternalInput")
    bk_d = nc.dram_tensor("bk", (H, Dh), F32, kind="ExternalInput")
    bv_d = nc.dram_tensor("bv", (H, Dh), F32, kind="ExternalInput")
    bo_d = nc.dram_tensor("bo", (D,), F32, kind="ExternalInput")
    out_d = nc.dram_tensor("out", (S, D), BF16, kind="ExternalOutput")
    dbg = {}
    if debug:
        dbg["qT"] = nc.dram_tensor("dbg_qT", (NPAIR, NP, S), BF16, kind="ExternalOutput")
        dbg["kT"] = nc.dram_tensor("dbg_kT", (NPAIR, NP, S), BF16, kind="ExternalOutput")
        dbg["v"] = nc.dram_tensor("dbg_v", (KT, NP, H * Dh), BF16, kind="ExternalOutput")
        dbg["zT"] = nc.dram_tensor("dbg_zT", (NPAIR, NP, S), BF16, kind="ExternalOutput")
        dbg["xTb"] = nc.dram_tensor("dbg_xTb", (DT, NP, S), BF16, kind="ExternalOutput")

    with tile.TileContext(nc) as tc:
        _body(tc, x8_d, r8_d, wq_d, wk_d, wv_d, wo_d, bq_d, bk_d, bv_d, bo_d,
              out_d, dbg)

    nc.compile()
    return nc


def _body(tc, x8_d, r8_d, wq_d, wk_d, wv_d, wo_d, bq_d, bk_d, bv_d, bo_d,
          out_d, dbg):
    nc = tc.nc

    with (
        tc.tile_pool(name="const", bufs=1) as const_pool,
        tc.tile_pool(name="qkT", bufs=1) as qkT_pool,
        tc.tile_pool(name="vsb", bufs=1) as v_pool,
        tc.tile_pool(name="zT", bufs=1) as zT_pool,
        tc.tile_pool(name="xT", bufs=1) as xT_pool,
        tc.tile_pool(name="w", bufs=1) as w_pool,
        tc.tile_pool(name="pt", bufs=30) as pt_pool,
        tc.tile_pool(name="rcp", bufs=2) as r_pool,
        tc.tile_pool(name="osb", bufs=3) as o_pool,
    ):
        # ---- constants (no DMA deps) ----
        warm = const_pool.tile([NP, NP], BF16, tag="warm")
        nc.gpsimd.memset(warm[:], 1.0)
        warm2 = const_pool.tile([NP, 256], BF16, tag="warm2")
        nc.gpsimd.memset(warm2[:], 1.0)
        ones64 = const_pool.tile([NP, 64], BF16, tag="ones64")
        nc.gpsimd.memset(ones64[:], 1.0)
        bq_sb = const_pool.tile([NP, NPAIR], F32, tag="bq")
        bk_sb = const_pool.tile([NP, NPAIR], F32, tag="bk")
        bv_rep = const_pool.tile([NP, H * Dh], F32, tag="bvrep")
        bo_rep = const_pool.tile([NP, D], F32, tag="borep")
        fill0 = nc.gpsimd.to_reg(0.0)

        # ---- persistent tiles ----
        qT = [qkT_pool.tile([NP, S], BF16, tag=f"qT{p}", name=f"qT{p}")
              for p in range(NPAIR)]
        kT = [qkT_pool.tile([NP, S], BF16, tag=f"kT{p}", name=f"kT{p}")
              for p in range(NPAIR)]
        v_sb = [v_pool.tile([NP, H * Dh], BF16, tag=f"v{k}", name=f"v{k}")
                for k in range(KT)]
        zT = [zT_pool.tile([NP, S], BF16, tag=f"zT{p}", name=f"zT{p}")
              for p in range(NPAIR)]
        x8_sb = xT_pool.tile([NP, DTP, 2, S], FP8, tag="x8", name="x8_sb")
        r8_sb = xT_pool.tile([NP, DTP, 2, S], FP8, tag="r8", name="r8_sb")
        xTb = [xT_pool.tile([NP, S], BF16, tag=f"xb{t}", name=f"xb{t}")
               for t in range(DT)]
        wk_sb = w_pool.tile([NP, NPAIR, DTP, 2, NP], FP8, tag="wk", name="wk_sb")
        wq_sb = w_pool.tile([NP, NPAIR, DTP, 2, NP], FP8, tag="wq", name="wq_sb")
        wv_sb = w_pool.tile([NP, DT, H * Dh], BF16, tag="wv", name="wv_sb")
        wo_sb = w_pool.tile([NP, NPAIR, D], BF16, tag="wo", name="wo_sb")

        # ---- DMA issues (per queue = priority order; all partition-major) ----
        # x8 gates everything: split it 3 ways so QK can start ~9us
        nc.sync.dma_start(x8_sb[:, 0:1], x8_d.ap()[:, 0:1])
        nc.scalar.dma_start(x8_sb[:, 1:2], x8_d.ap()[:, 1:2])
        nc.gpsimd.dma_start(x8_sb[:, 2:3], x8_d.ap()[:, 2:3])
        # sync: wk p0 (QK0), then wv halves
        nc.sync.dma_start(wk_sb[:, 0:1], wk_d.ap()[:, 0:1])
        nc.sync.dma_start(wv_sb[:, 0:3, :], wv_d.ap()[:, 0:3, :])
        nc.sync.dma_start(wv_sb[:, 3:6, :], wv_d.ap()[:, 3:6, :])
        # scalar: wk rest (QK1+), r8 (xTb/V)
        nc.scalar.dma_start(wk_sb[:, 1:NPAIR], wk_d.ap()[:, 1:NPAIR])
        nc.scalar.dma_start(r8_sb[:], r8_d.ap())
        # gpsimd: small biases + wq p0, bv broadcast, wq rest, wo, bo
        nc.gpsimd.dma_start(
            bq_sb[:], bq_d.ap().rearrange("h e -> (h e)").rearrange("(j p) -> p j", p=NP)
        )
        nc.gpsimd.dma_start(
            bk_sb[:], bk_d.ap().rearrange("h e -> (h e)").rearrange("(j p) -> p j", p=NP)
        )
        nc.gpsimd.dma_start(wq_sb[:, 0:1], wq_d.ap()[:, 0:1])
        nc.gpsimd.dma_start(
            bv_rep[:],
            bv_d.ap().rearrange("h e -> (h e)").unsqueeze(0).broadcast_to((NP, H * Dh)),
        )
        nc.gpsimd.dma_start(wq_sb[:, 1:NPAIR], wq_d.ap()[:, 1:NPAIR])
        nc.gpsimd.dma_start(wo_sb[:], wo_d.ap())
        nc.gpsimd.dma_start(bo_rep[:], bo_d.ap().unsqueeze(0).broadcast_to((NP, D)))

        with (
            tc.tile_pool(name="psQK", bufs=2, space="PSUM") as psQK,
            tc.tile_pool(name="psS", bufs=2, space="PSUM") as psS,
            tc.tile_pool(name="psZ", bufs=1, space="PSUM") as psZ,
            tc.tile_pool(name="psL", bufs=1, space="PSUM") as psL,
        ):
            # ---- PE warmup: ramp p-state while DMAs land ----
            wps = psQK.tile([NP, 512], F32, tag="qk", name="warmps")
            for i in range(N_WARM):
                nc.tensor.matmul(wps[:, 0:256], warm[:], warm2[:],
                                 start=True, stop=True, skip_group_check=True)

            def emit_qk_group(p, which):
                """Q or K projection for pair p: fp8 DoubleRow, 3 dtp x 2 sc,
                bf16 output with bias."""
                w_sb, b_sb, dstT = (
                    (wk_sb, bk_sb, kT) if which == 0 else (wq_sb, bq_sb, qT)
                )
                pss = [psQK.tile([NP, 512], F32, tag="qk",
                                 name=f"qk{p}_{which}_{sc}") for sc in range(2)]
                for tp in range(DTP):
                    lhs = w_sb[:, p, tp, :, :]
                    for sc in range(2):
                        nc.tensor.matmul(
                            pss[sc][:], lhs, x8_sb[:, tp, :, sc * 512:(sc + 1) * 512],
                            start=(tp == 0), stop=(tp == DTP - 1), perf_mode=DRSI,
                        )
                for sc in range(2):
                    nc.vector.tensor_scalar_add(
                        dstT[p][:, sc * 512:(sc + 1) * 512], pss[sc][:],
                        b_sb[:, p:p + 1]
                    )

            def block_geom(qh, kt):
                qlo = qh * 512
                q0 = kt * NP
                c0 = max(q0, qlo)
                w = qlo + 512 - c0
                return qlo, q0, c0, w

            def emit_score_block(p, qh, kt, pts):
                """S^T (bf16) + exp (+ causal diag mask) for one block."""
                qlo, q0, c0, w = block_geom(qh, kt)
                st = psS.tile([NP, 2, 512], F32, tag="st", name="st")
                for h in range(2):
                    nc.tensor.matmul(
                        st[:, h, 0:w],
                        kT[p][h * 64:(h + 1) * 64, q0:q0 + NP],
                        qT[p][h * 64:(h + 1) * 64, c0:c0 + w],
                        start=True, stop=True,
                    )
                pt = pt_pool.tile([NP, 2, 512], BF16, tag="pt", name="pt")
                nc.scalar.activation(pt[:, :, 0:w], st[:, :, 0:w], EXP, scale=SCALE)
                if c0 == q0:  # diagonal block: zero out k > q
                    nc.gpsimd.affine_select(
                        out=pt[:, 0:2, 0:NP], in_=pt[:, 0:2, 0:NP],
                        pattern=[[0, 2], [1, NP]],
                        compare_op=mybir.AluOpType.is_ge,
                        fill=fill0, base=0, channel_multiplier=-1,
                    )
                pts[kt] = (pt, c0, w)

            def block_list(qh):
                return list(range(4)) if qh == 0 else list(range(KT))

            def emit_scores(p, qh, pts):
                for kt in block_list(qh):
                    emit_score_block(p, qh, kt, pts)

            def emit_zl(p, qh, pts, filler=None):
                qlo = qh * 512
                z_ps = psZ.tile([NP, 512], F32, tag="z", name="z")
                l_ps = psL.tile([NP, 512], F32, tag="l", name="l")
                kts = block_list(qh)
                for kt in kts:
                    if filler is not None:
                        filler()
                    pt, c0, w = pts.pop(kt)
                    first = kt == kts[0]
                    last = kt == kts[-1]

                    def mm_l(h):
                        nc.tensor.matmul(
                            l_ps[h * 64:(h + 1) * 64, c0 - qlo:c0 - qlo + w],
                            ones64[:, 0:64], pt[:, h, 0:w],
                            start=first, stop=last, skip_group_check=True,
                        )

                    def mm_z(h):
                        nc.tensor.matmul(
                            z_ps[h * 64:(h + 1) * 64, c0 - qlo:c0 - qlo + w],
                            v_sb[kt][:, (2 * p + h) * 64:(2 * p + h + 1) * 64],
                            pt[:, h, 0:w],
                            start=first, stop=last, skip_group_check=True,
                        )
                    mm_l(0); mm_z(1); mm_l(1); mm_z(0)
                recip = r_pool.tile([NP, 512], F32, tag="rcp", name="recip")
                nc.vector.reciprocal_approx_fast(out=recip[:], in_=l_ps[:])
                nc.vector.tensor_mul(zT[p][:, qlo:qlo + 512], z_ps[:], recip[:])

            def emit_v(kt):
                ps1 = psQK.tile([NP, 512], F32, tag="qk", name=f"v{kt}a")
                ps2 = psQK.tile([NP, 512], F32, tag="qk", name=f"v{kt}b")
                for t in range(DT):
                    lhs = xTb[t][:, kt * NP:(kt + 1) * NP]
                    nc.tensor.matmul(ps1[:], lhs, wv_sb[:, t, 0:512],
                                     start=(t == 0), stop=(t == DT - 1))
                    nc.tensor.matmul(ps2[:, 0:256], lhs, wv_sb[:, t, 512:768],
                                     start=(t == 0), stop=(t == DT - 1))
                nc.vector.tensor_add(v_sb[kt][:, 0:512], ps1[:], bv_rep[:, 0:512])
                nc.vector.tensor_add(v_sb[kt][:, 512:768], ps2[:, 0:256],
                                     bv_rep[:, 512:768])

            def emit_outproj(i):
                ps1 = psQK.tile([NP, 512], F32, tag="qk", name=f"op{i}a")
                ps2 = psQK.tile([NP, 512], F32, tag="qk", name=f"op{i}b")
                for p2 in range(NPAIR):
                    lhs = zT[p2][:, i * NP:(i + 1) * NP]
                    nc.tensor.matmul(ps1[:], lhs, wo_sb[:, p2, 0:512],
                                     start=(p2 == 0), stop=(p2 == NPAIR - 1))
                    nc.tensor.matmul(ps2[:, 0:256], lhs, wo_sb[:, p2, 512:768],
                                     start=(p2 == 0), stop=(p2 == NPAIR - 1))
                o_t = o_pool.tile([NP, D], BF16, tag="o", name=f"ot{i}")
                nc.vector.tensor_add(o_t[:, 0:512], ps1[:], bo_rep[:, 0:512])
                nc.vector.tensor_add(o_t[:, 512:768], ps2[:, 0:256], bo_rep[:, 512:768])
                nc.sync.dma_start(out_d.ap()[i * NP:(i + 1) * NP, :], o_t[:])

            # ================= schedule =================
            # Block-granular software pipeline: a filler queue of future
            # score blocks / QK groups is woven one item per z/l step so the
            # ACT engine always has exp work queued.
            pts = {(p, qh): {} for p in range(NPAIR) for qh in range(2)}

            fill_q = []
            for p in range(1, NPAIR):
                if p >= 2:
                    fill_q.append(('qk', p, 0))
                    fill_q.append(('qk', p, 1))
                for qh in range(2):
                    for kt in block_list(qh):
                        fill_q.append(('s', p, qh, kt))

            def filler():
                if not fill_q:
                    return
                item = fill_q.pop(0)
                if item[0] == 'qk':
                    emit_qk_group(item[1], item[2])
                else:
                    _, p_, qh_, kt_ = item
                    emit_score_block(p_, qh_, kt_, pts[(p_, qh_)])

            emit_qk_group(0, 0)
            emit_qk_group(0, 1)
            emit_scores(0, 0, pts[(0, 0)])
            emit_scores(0, 1, pts[(0, 1)])
            emit_qk_group(1, 0)
            emit_qk_group(1, 1)
            # reconstruct bf16 x for the V path; split gpsimd/DVE queues
            for t in range(DT):
                tp, sub = divmod(t, 2)
                eng = nc.gpsimd if t % 2 == 0 else nc.vector
                eng.tensor_add(xTb[t][:], x8_sb[:, tp, sub, :],
                               r8_sb[:, tp, sub, :])

            for kt in range(KT):
                emit_v(kt)
                filler()

            for p in range(NPAIR):
                for qh in range(2):
                    if (p, qh) == (NPAIR - 1, 1):
                        # drain any remaining scores first, then let the
                        # first outproj half ride the final ACT bursts
                        while fill_q:
                            filler()
                        for i in range(4):
                            emit_outproj(i)
                    emit_zl(p, qh, pts[(p, qh)], filler)

        # ---- output projection second half (dedicated PSUM, pipelined) ----
        with tc.tile_pool(name="psO", bufs=3, space="PSUM") as psO:
            for i in range(4, ST):
                ps = psO.tile([NP, 1024], F32, tag="o", name="psO")
                for p2 in range(NPAIR):
                    lhs = zT[p2][:, i * NP:(i + 1) * NP]
                    nc.tensor.matmul(ps[:, 0:512], lhs, wo_sb[:, p2, 0:512],
                                     start=(p2 == 0), stop=(p2 == NPAIR - 1))
                    nc.tensor.matmul(ps[:, 512:768], lhs, wo_sb[:, p2, 512:768],
                                     start=(p2 == 0), stop=(p2 == NPAIR - 1))
                o_t = o_pool.tile([NP, D], BF16, tag="o", name=f"ot{i}")
                # split bias/DMA so the store of the first half overlaps the
                # bias-add of the second on the final tiles
                nc.vector.tensor_add(o_t[:, 0:384], ps[:, 0:384], bo_rep[:, 0:384])
                nc.sync.dma_start(out_d.ap()[i * NP:(i + 1) * NP, 0:384],
                                  o_t[:, 0:384])
                nc.vector.tensor_add(o_t[:, 384:768], ps[:, 384:768],
                                     bo_rep[:, 384:768])
                nc.sync.dma_start(out_d.ap()[i * NP:(i + 1) * NP, 384:768],
                                  o_t[:, 384:768])

        if dbg:
            for p in range(NPAIR):
                nc.sync.dma_start(dbg["qT"].ap()[p], qT[p][:])
                nc.sync.dma_start(dbg["kT"].ap()[p], kT[p][:])
                nc.sync.dma_start(dbg["zT"].ap()[p], zT[p][:])
            for kt in range(KT):
                nc.sync.dma_start(dbg["v"].ap()[kt], v_sb[kt][:])
            for t in range(DT):
                nc.sync.dma_start(dbg["xTb"].ap()[t], xTb[t][:])


_NC = None


def _get_nc():
    global _NC
    if _NC is None:
        nc = _build(debug=False)
        nc.m = get_hw_module(nc.m)
        _NC = nc
    return _NC


def _host_pack(inputs):
    import ml_dtypes
    E4 = ml_dtypes.float8_e4m3
    BF = ml_dtypes.bfloat16

    def pack_x(xb):
        # x [S, D] f32 -> xT [D, S] -> fp8 + fp8 residual, [NP, DTP, 2, S]
        xt = np.ascontiguousarray(xb.T).astype(np.float32)
        x8 = xt.astype(E4)
        r = xt - x8.astype(np.float32)
        r8 = r.astype(E4)

        def fold(a):
            return np.ascontiguousarray(
                a.reshape(DTP, 2, NP, S).transpose(2, 0, 1, 3))
        return fold(x8), fold(r8)

    def pack_wqk(w):
        # [H, D, Dh] -> [D, H*Dh] fp8 -> [NP, NPAIR, DTP, 2, NP] with the
        # DoubleRowSwInterleave layout: flat free index 2*(NP-1-j)+sub
        wt = np.asarray(w, np.float32).transpose(1, 0, 2).reshape(D, H * Dh)
        w8 = wt.astype(E4)
        a = w8.reshape(DTP, 2, NP, H * Dh).transpose(2, 0, 1, 3)  # [dp, tp, sub, he]
        a = a.reshape(NP, DTP, 2, NPAIR, NP).transpose(0, 3, 1, 2, 4)
        # [pair-major] -> interleave: [., ., ., sub, j] -> [., ., ., jrev, sub]
        a = a[..., ::-1].transpose(0, 1, 2, 4, 3)
        return np.ascontiguousarray(a)

    def pack_wv(w):
        # [H, D, Dh] -> [D, H*Dh] -> [NP(dp), DT, H*Dh] bf16
        wt = np.asarray(w, np.float32).transpose(1, 0, 2).reshape(DT, NP, H * Dh)
        return np.ascontiguousarray(wt.transpose(1, 0, 2)).astype(BF)

    wo = np.asarray(inputs["W_O"], np.float32)
    # [H, Dh, D] -> [NPAIR, NP(he), D] -> [NP(he), NPAIR, D]
    wo_p = np.ascontiguousarray(wo.reshape(NPAIR, NP, D).transpose(1, 0, 2))
    shared = {
        "wq8": pack_wqk(inputs["W_Q"]),
        "wk8": pack_wqk(inputs["W_K"]),
        "wv": pack_wv(inputs["W_V"]),
        "wo": wo_p.astype(BF),
        "bq": np.ascontiguousarray(np.asarray(inputs["b_Q"], np.float32)),
        "bk": np.ascontiguousarray(np.asarray(inputs["b_K"], np.float32)),
        "bv": np.ascontiguousarray(np.asarray(inputs["b_V"], np.float32)),
        "bo": np.ascontiguousarray(np.asarray(inputs["b_O"], np.float32)),
    }
    x = np.asarray(inputs["normalized_resid_pre"], np.float32)
    maps = []
    for b in range(B):
        x8, r8 = pack_x(x[b])
        maps.append(dict(shared, x8=x8, r8=r8))
    return maps


def kernel(**inputs):
    nc = _get_nc()
    res = bass_utils.run_bass_kernel_spmd(nc, _host_pack(inputs),
                                          core_ids=list(range(B)))
    return np.stack([res.results[b]["out"] for b in range(B)],
                    axis=0).astype(np.float32)


def kernel_traced(**inputs):
    """Like kernel() but captures an NTFF profile (ntff shim must be
    installed by the caller). Returns (out, BassKernelResults)."""
    nc = _get_nc()
    res = bass_utils.run_bass_kernel_spmd(
        nc, _host_pack(inputs), core_ids=list(range(B)), trace=True
    )
    out = np.stack([res.results[b]["out"] for b in range(B)],
                   axis=0).astype(np.float32)
    return out, res
